# revision 24
# baseline (speedup 1.0000x reference)
# Trainium2 Bass kernel for nn_BondLevel (gnn_message_passing).
#
# Sharding: data-parallel over the 16 graphs -> 2 graphs per NeuronCore,
# 8 cores, no collectives.  Per core:
#   init MLP factored per-atom:   pre0 = u[row] + v[col]; u,v = af @ iw1a/b
#   message MLP factored per-bond: pre = a[dst] + b[src],
#       a = x@W1a + emb_w[bt] + mb1  (emb_w = emb@W1c),   b = x@W1b
#   edges sorted by dst, grouped by dst-degree k, slot-major layout so the
#   segment sum is k-1 dense adds on DVE; b[src] via SWDGE dma_gather.
#   mw2 folded into uw1a on host: W_A = mw2@uw1a, bias_A = mb2@uw1a.
# Per-bond tensors live feature-major [64|65, NBP]; per-edge tensors live
# token-major (gather layout); PE transposes bridge the two.
import os
import numpy as np
import ml_dtypes

import concourse.bass as bass
import concourse.bacc as bacc
import concourse.tile as tile
import concourse.mybir as mybir
from concourse.bass_utils import run_bass_kernel_spmd

F32 = mybir.dt.float32
BF16 = mybir.dt.bfloat16
I16 = mybir.dt.int16

N, B, E, H, D, L = 30000, 16, 60000, 128, 64, 2
NCORES = 8
GPC = B // NCORES          # graphs per core
NG = N // B                # atoms per graph
NATOM = GPC * NG           # atoms per core
NATOMP = ((NATOM + 127) // 128) * 128
ZATOM = NATOM              # zero-row index in padded atom arrays


def _ceil(a, b):
    return (a + b - 1) // b


def _wrap16(idx):
    """[16, n/16] SWDGE index layout (idx t at [t%16, t//16]), replicated to
    128 partitions (8 copies, one per Q7 core)."""
    n = len(idx)
    assert n % 16 == 0
    w = np.asarray(idx, np.int16).reshape(n // 16, 16).T.copy()
    return np.ascontiguousarray(np.tile(w, (8, 1)))


_SIGMOID_MODE = bool(int(os.environ.get("KACT_SIGMOID", "0")))


def silu_np(x):
    if _SIGMOID_MODE:
        return 1.0 / (1.0 + np.exp(-x))
    return x / (1.0 + np.exp(-x))


def _plan(edge_index, bond_edge_index):
    """Common (SPMD-uniform) padded layout + per-core tables."""
    row, col = np.asarray(edge_index[0]), np.asarray(edge_index[1])
    src, dst = np.asarray(bond_edge_index[0]), np.asarray(bond_edge_index[1])
    g_bond = row // NG
    assert np.all(np.diff(g_bond) >= 0), "bonds must be sorted by graph"

    cores = []
    for c in range(NCORES):
        glo, ghi = GPC * c, GPC * (c + 1)
        b0 = int(np.searchsorted(g_bond, glo))
        b1 = int(np.searchsorted(g_bond, ghi))
        nb = b1 - b0
        esel = (dst >= b0) & (dst < b1)
        ls, ld = src[esel] - b0, dst[esel] - b0
        assert ls.min() >= 0 and ls.max() < nb, "bond edges cross graphs"
        deg = np.bincount(ld, minlength=nb)
        abase = GPC * NG * c
        cores.append(dict(b0=b0, b1=b1, nb=nb, ls=ls, ld=ld, deg=deg,
                          gb=g_bond[b0:b1] - glo,
                          r_loc=row[b0:b1] - abase, c_loc=col[b0:b1] - abase))

    kmax = max(int(co["deg"].max()) for co in cores)
    h0 = np.zeros(kmax + 1, np.int64)
    h1 = np.zeros(kmax + 1, np.int64)
    for co in cores:
        for k in range(kmax + 1):
            m = co["deg"] == k
            h0[k] = max(h0[k], int((m & (co["gb"] == 0)).sum()))
            h1[k] = max(h1[k], int((m & (co["gb"] == 1)).sum()))
    npad = np.array([_ceil(int(h0[k] + h1[k]), 128) * 128
                     for k in range(kmax + 1)])
    goff = np.concatenate([[0], np.cumsum(npad)])
    NBP = int(goff[-1])

    groups = []
    for k in range(1, kmax + 1):
        if npad[k] == 0:
            continue
        chunks = int(npad[k]) // 128
        c_sub = max(1, min(32 // k, chunks))
        while chunks % c_sub:
            c_sub -= 1
        groups.append(dict(k=k, off=int(goff[k]), chunks=chunks, c_sub=c_sub))

    for co in cores:
        nb, deg, gb = co["nb"], co["deg"], co["gb"]
        pad_of = np.full(NBP, -1, np.int64)
        pos_of = np.full(nb, -1, np.int64)
        for k in range(kmax + 1):
            for side, base in ((0, 0), (1, int(h0[k]))):
                ids = np.nonzero((deg == k) & (gb == side))[0]
                p = goff[k] + base + np.arange(len(ids))
                pad_of[p] = ids
                pos_of[ids] = p
        co["pad_of"], co["pos_of"] = pad_of, pos_of
        real = pad_of >= 0

        ridx = np.full(NBP, ZATOM, np.int64)
        cidx = np.full(NBP, ZATOM, np.int64)
        ridx[real] = co["r_loc"][pad_of[real]]
        cidx[real] = co["c_loc"][pad_of[real]]
        co["ridx_t"], co["cidx_t"] = _wrap16(ridx), _wrap16(cidx)

        dinv = np.zeros(NBP, np.float32)
        mask = np.zeros(NBP, np.float32)
        dd = deg[pad_of[real]]
        dinv[real] = 1.0 / np.maximum(dd, 1.0)
        mask[real] = (dd > 0).astype(np.float32)
        co["dinv_tok"] = np.ascontiguousarray(
            dinv.reshape(NBP // 128, 128).T).astype(np.float32)
        co["mask_row"] = np.ascontiguousarray(mask.reshape(1, NBP)).astype(ml_dtypes.bfloat16)

        order = np.argsort(co["ld"], kind="stable")
        ls_s, ld_s = co["ls"][order], co["ld"][order]
        bounds = np.searchsorted(ld_s, np.arange(nb + 1))
        tab = []
        for g in groups:
            k, off, chunks, c_sub = g["k"], g["off"], g["chunks"], g["c_sub"]
            for blk in range(chunks // c_sub):
                i0 = blk * c_sub * 128
                sub = np.full((k, c_sub * 128), NBP, np.int64)
                for t in range(c_sub * 128):
                    ob = pad_of[off + i0 + t]
                    if ob >= 0 and deg[ob] == k:
                        ss = ls_s[bounds[ob]:bounds[ob + 1]]
                        sub[:, t] = pos_of[ss]
                tab.append(sub.reshape(-1))
        tab = np.concatenate(tab) if tab else np.zeros(16, np.int64)
        co["src_tab"] = _wrap16(tab)
        co["cnt"] = np.array([(gb == 0).sum(), (gb == 1).sum()], np.float64)

    spans = []  # (graph, start, len) common across cores
    for k in range(kmax + 1):
        if npad[k] == 0:
            continue
        o = int(goff[k])
        if h0[k]:
            spans.append((0, o, int(h0[k])))
        if h1[k]:
            spans.append((1, o + int(h0[k]), int(h1[k])))
    TOT = sum(g["k"] * g["chunks"] * 128 for g in groups)
    deg0span = (int(goff[0]), int(npad[0])) if npad[0] else None
    return dict(NBP=NBP, groups=groups, spans=spans, deg0span=deg0span,
                kmax=kmax, TOT=TOT, cores=cores)


def _weights(inp):
    iw1, ib1 = np.asarray(inp["iw1"], np.float32), np.asarray(inp["ib1"], np.float32)
    iw2, ib2 = np.asarray(inp["iw2"], np.float32), np.asarray(inp["ib2"], np.float32)
    emb = np.asarray(inp["emb"], np.float32)
    mw1, mb1 = np.asarray(inp["mw1"], np.float32), np.asarray(inp["mb1"], np.float32)
    mw2, mb2 = np.asarray(inp["mw2"], np.float32), np.asarray(inp["mb2"], np.float32)
    uw1, ub1 = np.asarray(inp["uw1"], np.float32), np.asarray(inp["ub1"], np.float32)
    uw2, ub2 = np.asarray(inp["uw2"], np.float32), np.asarray(inp["ub2"], np.float32)

    w = {}
    w["rhs_uv"] = np.ascontiguousarray(
        np.concatenate([iw1[:H], iw1[H:]], axis=1))          # [128, 128]
    w["ib1_half"] = np.concatenate([ib1 / 2, ib1 / 2]).reshape(1, 2 * D)
    w["has_ib1"] = bool(np.any(ib1 != 0))
    w["lhsT_iw2"] = np.concatenate([iw2, np.zeros((1, D), np.float32)]).astype(ml_dtypes.bfloat16)
    w["ib2"] = np.ascontiguousarray(ib2.reshape(D, 1))
    te_tab = np.zeros((8, 2 * D), np.float32)
    rhs_ab, lhsT_WA, lhsT_uw1b, lhsT_uw2, lhsT_I = [], [], [], [], []
    for l in range(L):
        W1a, W1b, W1c = mw1[l][:D], mw1[l][D:2 * D], mw1[l][2 * D:]
        te_tab[:5, l * D:(l + 1) * D] = emb[l] @ W1c
        ra = np.zeros((D + 1, 2 * D), np.float32)
        ra[:D, :D], ra[D, :D] = W1a, mb1[l]
        ra[:D, D:] = W1b
        rhs_ab.append(ra)
        WA = mw2[l] @ uw1[l][:D]
        bA = (mb2[l] @ uw1[l][:D]).reshape(1, D)
        lhsT_WA.append(np.concatenate([WA, bA]))
        lhsT_uw1b.append(np.concatenate([uw1[l][D:], np.zeros((1, D), np.float32)]))
        lhsT_uw2.append(np.concatenate([uw2[l], np.zeros((1, D), np.float32)]))
        lhsT_I.append(np.concatenate([np.eye(D, dtype=np.float32),
                                      np.zeros((1, D), np.float32)]))
    w["te_tab"] = te_tab.astype(ml_dtypes.bfloat16)
    w["rhs_ab"] = np.ascontiguousarray(np.stack(rhs_ab))
    w["lhsT_WA"] = np.ascontiguousarray(np.stack(lhsT_WA)).astype(ml_dtypes.bfloat16)
    w["lhsT_uw1b"] = np.ascontiguousarray(np.stack(lhsT_uw1b))
    w["lhsT_uw2"] = np.ascontiguousarray(np.stack(lhsT_uw2)).astype(ml_dtypes.bfloat16)
    w["lhsT_I"] = np.ascontiguousarray(np.stack(lhsT_I))
    w["ub1"] = np.ascontiguousarray(ub1.reshape(L, D, 1))
    w["ub2"] = np.ascontiguousarray(ub2.reshape(L, D, 1))
    w["ident"] = np.eye(128, dtype=np.float32)

    # value of padded x columns per layer end (depends only on biases)
    xp = silu_np(np.zeros(D, np.float32) + ib1) @ iw2 + ib2
    for l in range(L):
        hp = silu_np(xp @ uw1[l][D:] + ub1[l])
        xp = xp + hp @ uw2[l] + ub2[l]
    w["x_padval"] = xp.astype(np.float32).astype(np.float64)
    return w


def _build(plan, w):
    NBP = plan["NBP"]
    CB = NBP // 128
    TOT = plan["TOT"]
    TOTP = max(TOT, 256)
    sweeps = [(s, min(512, NBP - s)) for s in range(0, NBP, 512)]
    nc = bacc.Bacc("TRN2", target_bir_lowering=False, debug=False,
                   num_devices=NCORES)

    def din(name, shape, dt):
        return nc.dram_tensor(name, list(shape), dt, kind="ExternalInput")

    i_af = din("af", (NATOMP, H), F32)
    i_ridx = din("ridx", (128, NBP // 16), I16)
    i_cidx = din("cidx", (128, NBP // 16), I16)
    i_stab = din("stab", (128, TOTP // 16), I16)
    i_dinv = din("dinv", (128, CB), F32)
    i_mask = din("maskrow", (1, NBP), BF16)
    i_rhs_uv = din("rhs_uv", (H, 2 * D), F32)
    i_lhsT_iw2 = din("lhsT_iw2", (D + 1, D), BF16)
    i_ib2 = din("ib2", (D, 1), F32)
    i_te_tab = din("te_tab", (8, 2 * D), BF16)
    i_rhs_ab = din("rhs_ab", (L, D + 1, 2 * D), F32)
    i_lhsT_WA = din("lhsT_WA", (L, D + 1, D), BF16)
    i_lhsT_uw1b = din("lhsT_uw1b", (L, D + 1, D), F32)
    i_lhsT_uw2 = din("lhsT_uw2", (L, D + 1, D), BF16)
    i_lhsT_I = din("lhsT_I", (L, D + 1, D), F32)
    i_ub1 = din("ub1", (L, D, 1), F32)
    i_ub2 = din("ub2", (L, D, 1), F32)
    i_ident = din("ident", (128, 128), F32)
    if w["has_ib1"]:
        i_ib1h = din("ib1_half", (1, 2 * D), F32)

    o_x = nc.dram_tensor("x_out", [D, NBP], F32, kind="ExternalOutput")
    o_gf = nc.dram_tensor("gf_out", [D, GPC], F32, kind="ExternalOutput")

    SILU = (mybir.ActivationFunctionType.Sigmoid if _SIGMOID_MODE
            else mybir.ActivationFunctionType.Silu)
    IDENT = mybir.ActivationFunctionType.Identity
    SQRT = mybir.ActivationFunctionType.Sqrt
    ALU = mybir.AluOpType
    AX = mybir.AxisListType
    BLK = 8

    with tile.TileContext(nc) as tc:
        with (
            tc.tile_pool(name="persist", bufs=1) as pp,
            tc.tile_pool(name="weights", bufs=1) as wp,
            tc.tile_pool(name="psA", bufs=2, space="PSUM") as psA,
            tc.tile_pool(name="psT", bufs=2, space="PSUM") as psT,
            tc.tile_pool(name="psU", bufs=3, space="PSUM") as psU,
            tc.tile_pool(name="dram", bufs=1, space="DRAM") as dp,
        ):
            x_T = pp.tile([D + 1, NBP], F32, tag="x_T")
            s_T = pp.tile([D + 1, NBP], BF16, tag="s_T")
            a_tok = pp.tile([128, CB, D], BF16, tag="a_tok")
            te_sel = pp.tile([128, CB, 2 * D], BF16, tag="te_sel")
            stab = pp.tile([128, TOTP // 16], I16, tag="stab")
            ridx = pp.tile([128, NBP // 16], I16, tag="ridx")
            cidx = pp.tile([128, NBP // 16], I16, tag="cidx")
            te_idx = pp.tile([128, NBP // 16], I16, tag="te_idx")
            dinv = pp.tile([128, CB], F32, tag="dinv")
            ident = pp.tile([128, 128], F32, tag="ident")
            btf = pp.tile([128, CB], F32, tag="btf")
            bti = pp.tile([128, CB], I16, tag="bti")
            sdot = pp.tile([128, CB], F32, tag="sdot")
            ni2 = pp.tile([128, CB], F32, tag="ni2")
            nj2 = pp.tile([128, CB], F32, tag="nj2")
            partials = pp.tile([D, 64], F32, tag="partials")

            rhs_uv = wp.tile([H, 2 * D], F32, tag="w0")
            lhsT_iw2 = wp.tile([D + 1, D], BF16, tag="w1")
            ib2 = wp.tile([D, 1], F32, tag="w2")
            rhs_ab = [wp.tile([D + 1, 2 * D], F32, tag=f"wab{l}", name=f"rhs_ab{l}") for l in range(L)]
            lhsT_WA = [wp.tile([D + 1, D], BF16, tag=f"wWA{l}", name=f"lhsT_WA{l}") for l in range(L)]
            lhsT_uw1b = [wp.tile([D + 1, D], F32, tag=f"wu1{l}", name=f"lhsT_uw1b{l}") for l in range(L)]
            lhsT_uw2 = [wp.tile([D + 1, D], BF16, tag=f"wu2{l}", name=f"lhsT_uw2{l}") for l in range(L)]
            lhsT_I = [wp.tile([D + 1, D], F32, tag=f"wI{l}", name=f"lhsT_I{l}") for l in range(L)]
            ub1 = [wp.tile([D, 1], F32, tag=f"b1{l}", name=f"ub1_{l}") for l in range(L)]
            ub2 = [wp.tile([D, 1], F32, tag=f"b2{l}", name=f"ub2_{l}") for l in range(L)]

            for t, srcap in ((rhs_uv, i_rhs_uv), (lhsT_iw2, i_lhsT_iw2),
                             (ib2, i_ib2), (ident, i_ident), (stab, i_stab),
                             (ridx, i_ridx), (cidx, i_cidx), (dinv, i_dinv)):
                nc.sync.dma_start(t[:], srcap[:])
            for l in range(L):
                for t, srcap in ((rhs_ab[l], i_rhs_ab), (lhsT_WA[l], i_lhsT_WA),
                                 (lhsT_uw1b[l], i_lhsT_uw1b),
                                 (lhsT_uw2[l], i_lhsT_uw2), (lhsT_I[l], i_lhsT_I),
                                 (ub1[l], i_ub1), (ub2[l], i_ub2)):
                    nc.sync.dma_start(t[:], srcap[l])
            nc.sync.dma_start(s_T[D:D + 1, :], i_mask[:])
            nc.vector.memset(x_T[D:D + 1, :], 1.0)

            ubuf = dp.tile([NATOMP, D], F32, tag="ubuf")
            vbuf = dp.tile([NATOMP, D], F32, tag="vbuf")
            bbuf = dp.tile([NBP + 16, D], F32, tag="bbuf")
            te_hbm = dp.tile([8, 2 * D], BF16, tag="tehbm")
            nc.sync.dma_start(te_hbm[:], i_te_tab[:])

            # ---- prologue --------------------------------------------------
            ACH = NATOMP // 128
            if w["has_ib1"]:
                ib1h = wp.tile([1, 2 * D], F32, tag="ib1h")
                nc.sync.dma_start(ib1h[:], i_ib1h[:])
                ones_row = wp.tile([1, 128], F32, tag="ones_row")
                nc.vector.memset(ones_row[:], 1.0)
            # af -> af_T (PE transpose, 128-chunks), then u/v and write out
            p1 = tc.alloc_tile_pool(name="prolog", bufs=1)
            p2 = tc.alloc_tile_pool(name="prolog2", bufs=2)
            af_T = p1.tile([H, NATOMP], F32, tag="af_T")
            for b0 in range(0, ACH, BLK):
                nch = min(BLK, ACH - b0)
                afc = p2.tile([128, BLK, H], F32, tag="afc")
                nc.sync.dma_start(
                    afc[:, 0:nch, :],
                    i_af.ap().rearrange("(c p) f -> p c f", p=128)[:, b0:b0 + nch, :])
                for t0 in range(0, nch, 4):
                    nb4 = min(4, nch - t0)
                    pt = psT.tile([128, 512], F32, tag="ptb")
                    for i in range(nb4):
                        nc.tensor.transpose(pt[:, i * 128:(i + 1) * 128],
                                            afc[:, t0 + i, :], ident[:])
                    nc.scalar.activation(
                        af_T[:, (b0 + t0) * 128:(b0 + t0 + nb4) * 128],
                        pt[:, 0:nb4 * 128], IDENT)
                uvs = p2.tile([128, BLK, 2 * D], F32, tag="uvs")
                for t in range(nch):
                    cix = b0 + t
                    pab = psA.tile([128, 2 * D], F32, tag="pab")
                    nc.tensor.matmul(pab[:], af_T[:, cix * 128:(cix + 1) * 128],
                                     rhs_uv[:], start=True, stop=not w["has_ib1"])
                    if w["has_ib1"]:
                        nc.tensor.matmul(pab[:], ones_row[:], ib1h[:],
                                         start=False, stop=True)
                    nc.scalar.activation(uvs[:, t, :], pab[:], IDENT)
                nc.sync.dma_start(
                    ubuf[:, :].rearrange("(c p) f -> p c f", p=128)[:, b0:b0 + nch, :],
                    uvs[:, 0:nch, 0:D])
                nc.sync.dma_start(
                    vbuf[:, :].rearrange("(c p) f -> p c f", p=128)[:, b0:b0 + nch, :],
                    uvs[:, 0:nch, D:2 * D])

            p2.release()
            p1.release()
            gp = tc.alloc_tile_pool(name="gath", bufs=3)
            gpp = tc.alloc_tile_pool(name="gathp", bufs=2)
            sp = tc.alloc_tile_pool(name="stage", bufs=3)
            zrow = sp.tile([16, D], F32, tag="zrow")
            nc.vector.memset(zrow[:], 0.0)
            nc.sync.dma_start(bbuf[NBP:NBP + 16, :], zrow[:])

            # sim dot products + init-MLP gathers per 8-chunk block
            for blk in range(_ceil(CB, BLK)):
                c0 = blk * BLK
                nch = min(BLK, CB - c0)
                nidx = nch * 128
                hi = gpp.tile([128, BLK, H], F32, tag="gh0")
                hj = gpp.tile([128, BLK, H], F32, tag="gh1")
                nc.gpsimd.dma_gather(hi[:, 0:nch, :], i_af[:],
                                     ridx[:, c0 * 8:(c0 + nch) * 8],
                                     nidx, nidx, H, elem_step=H)
                nc.gpsimd.dma_gather(hj[:, 0:nch, :], i_af[:],
                                     cidx[:, c0 * 8:(c0 + nch) * 8],
                                     nidx, nidx, H, elem_step=H)
                scr = sp.tile([128, H], F32, tag="scr")
                for t in range(nch):
                    cc = c0 + t
                    nc.vector.tensor_tensor_reduce(
                        scr[:], hi[:, t, :], hj[:, t, :], 1.0, 0.0,
                        ALU.mult, ALU.add, sdot[:, cc:cc + 1])
                    nc.vector.tensor_tensor_reduce(
                        scr[:], hi[:, t, :], hi[:, t, :], 1.0, 0.0,
                        ALU.mult, ALU.add, ni2[:, cc:cc + 1])
                    nc.vector.tensor_tensor_reduce(
                        scr[:], hj[:, t, :], hj[:, t, :], 1.0, 0.0,
                        ALU.mult, ALU.add, nj2[:, cc:cc + 1])
                gu = gpp.tile([128, BLK, D], F32, tag="gg0")
                gv = gpp.tile([128, BLK, D], F32, tag="gg1")
                nc.gpsimd.dma_gather(gu[:, 0:nch, :], ubuf[:],
                                     ridx[:, c0 * 8:(c0 + nch) * 8],
                                     nidx, nidx, D, elem_step=D)
                nc.gpsimd.dma_gather(gv[:, 0:nch, :], vbuf[:],
                                     cidx[:, c0 * 8:(c0 + nch) * 8],
                                     nidx, nidx, D, elem_step=D)
                nc.vector.tensor_add(gu[:, 0:nch, :], gu[:, 0:nch, :],
                                     gv[:, 0:nch, :])
                nc.scalar.activation(gu[:, 0:nch, :], gu[:, 0:nch, :], SILU)
                for t in range(nch):
                    cc = c0 + t
                    pt2 = psT.tile([128, 128], F32, tag="pt")
                    nc.tensor.transpose(pt2[0:D, :], gu[:, t, 0:D], ident[:])
                    nc.scalar.activation(
                        s_T[0:D, cc * 128:(cc + 1) * 128], pt2[0:D, :], IDENT)

            # sim -> bt -> te_idx -> te gather
            nc.scalar.activation(ni2[:], ni2[:], SQRT)
            nc.scalar.activation(nj2[:], nj2[:], SQRT)
            nc.vector.tensor_scalar_max(ni2[:], ni2[:], 1e-8)
            nc.vector.tensor_scalar_max(nj2[:], nj2[:], 1e-8)
            nc.vector.tensor_mul(ni2[:], ni2[:], nj2[:])
            nc.vector.reciprocal(ni2[:], ni2[:])
            nc.vector.tensor_mul(sdot[:], sdot[:], ni2[:])   # sim
            is1, is2 = ni2, nj2
            nc.vector.tensor_scalar(is1[:], sdot[:], 0.8, None, ALU.is_gt)
            nc.vector.tensor_scalar(is2[:], sdot[:], 0.9, None, ALU.is_gt)
            nc.vector.tensor_scalar(btf[:], sdot[:], 0.3, None, ALU.is_lt)
            nc.vector.tensor_add(is1[:], is1[:], is2[:])
            nc.vector.tensor_scalar(is2[:], btf[:], -1.0, 1.0, ALU.mult, ALU.add)
            nc.vector.tensor_mul(is1[:], is1[:], is2[:])
            nc.vector.tensor_scalar(btf[:], btf[:], 3.0, None, ALU.mult)
            nc.vector.tensor_add(btf[:], btf[:], is1[:])
            nc.vector.tensor_copy(bti[:], btf[:])
            for r in range(8):
                nc.sync.dma_start(te_idx[0:16, r::8], bti[r * 16:(r + 1) * 16, :])
            for m in range(1, 8):
                nc.sync.dma_start(te_idx[m * 16:(m + 1) * 16, :], te_idx[0:16, :])
            half_c = CB // 2
            nc.gpsimd.dma_gather(te_sel[:, 0:half_c, :], te_hbm[:],
                                 te_idx[:, 0:half_c * 8], half_c * 128,
                                 half_c * 128, 2 * D, elem_step=2 * D)
            nc.gpsimd.dma_gather(te_sel[:, half_c:CB, :], te_hbm[:],
                                 te_idx[:, half_c * 8:], NBP - half_c * 128,
                                 NBP - half_c * 128, 2 * D, elem_step=2 * D)

            # x0 = silu(pre0) @ iw2 + ib2
            for s0, ln in sweeps:
                sl = slice(s0, s0 + ln)
                pu = psU.tile([D, 512], F32, tag="pu")
                nc.tensor.matmul(pu[0:D, 0:ln], lhsT_iw2[:], s_T[:, sl],
                                 start=True, stop=True)
                nc.scalar.activation(x_T[0:D, sl], pu[0:D, 0:ln], IDENT,
                                     bias=ib2[:])

            # ---- layers ----------------------------------------------------
            for l in range(L):
                for b0 in range(0, CB, BLK):
                    nch = min(BLK, CB - b0)
                    bst = sp.tile([128, BLK, D], F32, tag="bst")
                    for t in range(nch):
                        cix = b0 + t
                        pab = psA.tile([128, 2 * D], F32, tag="pab")
                        nc.tensor.matmul(pab[:],
                                         x_T[:, cix * 128:(cix + 1) * 128],
                                         rhs_ab[l][:], start=True, stop=True)
                        nc.vector.tensor_add(a_tok[:, cix, :], pab[:, 0:D],
                                             te_sel[:, cix, l * D:(l + 1) * D])
                        nc.scalar.activation(bst[:, t, :], pab[:, D:2 * D], IDENT)
                    nc.sync.dma_start(
                        bbuf[0:NBP, :].rearrange("(c p) f -> p c f",
                                                 p=128)[:, b0:b0 + nch, :],
                        bst[:, 0:nch, :])

                if plan["deg0span"] is not None:
                    o0, n0 = plan["deg0span"]
                    nc.vector.memset(s_T[0:D, o0:o0 + n0], 0.0)

                toff = 0
                for g in plan["groups"]:
                    k, off, chunks, c_sub = (g["k"], g["off"], g["chunks"],
                                             g["c_sub"])
                    for blk in range(chunks // c_sub):
                        nidx = k * c_sub * 128
                        gch = off // 128 + blk * c_sub
                        tg = gp.tile([128, 32, D], F32, tag="gath")
                        tgv = tg[:, 0:k * c_sub, :]
                        nc.gpsimd.dma_gather(
                            tgv, bbuf[:],
                            stab[:, toff // 16:(toff + nidx) // 16],
                            nidx, nidx, D, elem_step=D)
                        toff += nidx
                        for j in range(k):
                            nc.vector.tensor_add(
                                tg[:, j * c_sub:(j + 1) * c_sub, :],
                                tg[:, j * c_sub:(j + 1) * c_sub, :],
                                a_tok[:, gch:gch + c_sub, :])
                        nc.scalar.activation(tgv, tgv, SILU)
                        for j in range(1, k):
                            nc.vector.tensor_add(
                                tg[:, 0:c_sub, :], tg[:, 0:c_sub, :],
                                tg[:, j * c_sub:(j + 1) * c_sub, :])
                        for cc in range(c_sub):
                            nc.vector.tensor_scalar_mul(
                                tg[:, cc, :], tg[:, cc, :],
                                dinv[:, gch + cc:gch + cc + 1])
                            pt3 = psT.tile([128, 128], F32, tag="pt")
                            nc.tensor.transpose(pt3[0:D, :], tg[:, cc, 0:D],
                                                ident[:])
                            nc.scalar.activation(
                                s_T[0:D, (gch + cc) * 128:(gch + cc + 1) * 128],
                                pt3[0:D, :], IDENT)

                for s0, ln in sweeps:
                    sl = slice(s0, s0 + ln)
                    ph = psU.tile([D, 512], F32, tag="pu")
                    nc.tensor.matmul(ph[0:D, 0:ln], lhsT_WA[l][:], s_T[:, sl],
                                     start=True, stop=False)
                    nc.tensor.matmul(ph[0:D, 0:ln], lhsT_uw1b[l][:], x_T[:, sl],
                                     start=False, stop=True)
                    nc.scalar.activation(s_T[0:D, sl], ph[0:D, 0:ln], SILU,
                                         bias=ub1[l][:])
                    px = psU.tile([D, 512], F32, tag="pu")
                    nc.tensor.matmul(px[0:D, 0:ln], lhsT_uw2[l][:], s_T[:, sl],
                                     start=True, stop=False)
                    nc.tensor.matmul(px[0:D, 0:ln], lhsT_I[l][:], x_T[:, sl],
                                     start=False, stop=True)
                    nc.scalar.activation(x_T[0:D, sl], px[0:D, 0:ln], IDENT,
                                         bias=ub2[l][:])

            # ---- epilogue: pooling span sums + outputs --------------------
            nc.sync.dma_start(o_x.ap(), x_T[0:D, :])
            g0 = [(s, ln) for (g, s, ln) in plan["spans"] if g == 0]
            g1 = [(s, ln) for (g, s, ln) in plan["spans"] if g == 1]
            assert len(g0) <= 32 and len(g1) <= 32
            for i, (s, ln) in enumerate(g0):
                nc.vector.reduce_sum(partials[:, i:i + 1], x_T[0:D, s:s + ln],
                                     axis=AX.X)
            for i, (s, ln) in enumerate(g1):
                nc.vector.reduce_sum(partials[:, 32 + i:33 + i],
                                     x_T[0:D, s:s + ln], axis=AX.X)
            gf = sp.tile([D, GPC], F32, tag="gf")
            nc.vector.reduce_sum(gf[:, 0:1], partials[:, 0:len(g0)], axis=AX.X)
            nc.vector.reduce_sum(gf[:, 1:2], partials[:, 32:32 + len(g1)],
                                 axis=AX.X)
            nc.sync.dma_start(o_gf.ap(), gf[:])
            sp.release()
            gpp.release()
            gp.release()

    nc.compile()
    return nc


def _in_map(plan, w, core):
    co = plan["cores"][core]
    m = {
        "af": co["af_pad"], "ridx": co["ridx_t"], "cidx": co["cidx_t"],
        "stab": co["src_tab_pad"], "dinv": co["dinv_tok"],
        "maskrow": co["mask_row"],
        "rhs_uv": w["rhs_uv"], "lhsT_iw2": w["lhsT_iw2"], "ib2": w["ib2"],
        "te_tab": w["te_tab"], "rhs_ab": w["rhs_ab"],
        "lhsT_WA": w["lhsT_WA"], "lhsT_uw1b": w["lhsT_uw1b"],
        "lhsT_uw2": w["lhsT_uw2"], "lhsT_I": w["lhsT_I"],
        "ub1": w["ub1"], "ub2": w["ub2"], "ident": w["ident"],
    }
    if w["has_ib1"]:
        m["ib1_half"] = w["ib1_half"]
    return {k: np.ascontiguousarray(v) for k, v in m.items()}


class _SimRes:
    pass


def _pjrt_runner(nc, in_maps):
    """Build a reusable sharded PJRT callable for timing loops (the
    axon NTFF hook is unavailable in this container, so exec time is
    measured as steady-state wall time of the compiled executable)."""
    import jax
    import jax.numpy as jnp
    from jax.sharding import Mesh, PartitionSpec
    from jax.experimental.shard_map import shard_map
    from concourse import bass2jax
    import concourse.mybir as mybir
    bass2jax.install_neuronx_cc_hook()
    n_cores = len(in_maps)
    partition_name = (nc.partition_id_tensor.name
                      if nc.partition_id_tensor else None)
    in_names, out_names, out_avals, zero_outs = [], [], [], []
    for alloc in nc.m.functions[0].allocations:
        if not isinstance(alloc, mybir.MemoryLocationSet):
            continue
        name = alloc.memorylocations[0].name
        if alloc.kind == "ExternalInput":
            if name != partition_name:
                in_names.append(name)
        elif alloc.kind == "ExternalOutput":
            shape = tuple(alloc.tensor_shape)
            dtype = mybir.dt.np(alloc.dtype)
            out_names.append(name)
            out_avals.append(jax.core.ShapedArray(shape, dtype))
            zero_outs.append(np.zeros(shape, dtype))
    n_params = len(in_names)
    n_outs = len(out_avals)
    all_names = list(in_names) + list(out_names)
    if partition_name is not None:
        all_names.append(partition_name)

    def _body(*args):
        operands = list(args)
        if partition_name is not None:
            operands.append(bass2jax.partition_id_tensor())
        outs = bass2jax._bass_exec_p.bind(
            *operands, out_avals=tuple(out_avals), in_names=tuple(all_names),
            out_names=tuple(out_names), lowering_input_output_aliases=(),
            sim_require_finite=True, sim_require_nnan=True, nc=nc)
        return tuple(outs)

    devices = jax.devices()[:n_cores]
    mesh = Mesh(np.asarray(devices), ("core",))
    sharded = jax.jit(
        shard_map(_body, mesh=mesh,
                  in_specs=(PartitionSpec("core"),) * (n_params + n_outs),
                  out_specs=(PartitionSpec("core"),) * n_outs,
                  check_rep=False),
        keep_unused=True)
    from jax.sharding import NamedSharding
    shard = NamedSharding(mesh, PartitionSpec("core"))
    concat_in = [jax.device_put(
                     np.concatenate([np.asarray(in_maps[c][in_names[i]])
                                     for c in range(n_cores)], axis=0), shard)
                 for i in range(n_params)]
    concat_zeros = [jax.device_put(
                        np.zeros((n_cores * z.shape[0], *z.shape[1:]), z.dtype),
                        shard)
                    for z in zero_outs]
    jax.block_until_ready(concat_in)
    jax.block_until_ready(concat_zeros)

    nrep = int(os.environ.get("KNREP", "1"))
    if nrep > 1:
        # chain a scalar data-dependency through reps so XLA can't CSE or
        # parallelize the repeated NEFF executions
        def _body_n(*args):
            ins = list(args[:n_params])
            zouts = list(args[n_params:])
            outs = None
            for _ in range(nrep):
                operands = list(ins) + list(zouts)
                if partition_name is not None:
                    operands.append(bass2jax.partition_id_tensor())
                outs = bass2jax._bass_exec_p.bind(
                    *operands, out_avals=tuple(out_avals),
                    in_names=tuple(all_names), out_names=tuple(out_names),
                    lowering_input_output_aliases=(),
                    sim_require_finite=True, sim_require_nnan=True, nc=nc)
                ins[0] = ins[0] + (0.0 * outs[0].reshape(-1)[0]).astype(
                    ins[0].dtype)
            return tuple(outs)

        sharded_n = jax.jit(
            shard_map(_body_n, mesh=mesh,
                      in_specs=(PartitionSpec("core"),) * (n_params + n_outs),
                      out_specs=(PartitionSpec("core"),) * n_outs,
                      check_rep=False),
            keep_unused=True)

        def run():
            out = sharded_n(*concat_in, *concat_zeros)
            jax.block_until_ready(out)
            return out
    else:
        def run():
            out = sharded(*concat_in, *concat_zeros)
            jax.block_until_ready(out)
            return out

    def unpack(out_arrs):
        return [{name: np.asarray(out_arrs[i]).reshape(
                    n_cores, *out_avals[i].shape)[c]
                 for i, name in enumerate(out_names)}
                for c in range(n_cores)]

    return run, unpack


def _run_sim(nc, in_maps):
    from concourse.bass_interp import CoreSim
    results = []
    ncse = int(os.environ.get("KSIM_CORES", str(NCORES)))
    for m in in_maps[:ncse]:
        sim = CoreSim(nc, trace=False)
        for k, v in m.items():
            sim.tensor(k)[:] = v
        sim.simulate()
        results.append({o: np.array(sim.tensor(o))
                        for o in ("x_out", "gf_out")})
    while len(results) < NCORES:
        results.append(results[-1])
    r = _SimRes()
    r.results = results
    r.exec_time_ns = None
    return r


def kernel(**inputs):
    inp = {k: np.asarray(v) for k, v in inputs.items()}
    af = np.asarray(inp["atom_features"], np.float32)

    plan = _plan(inp["edge_index"], inp["bond_edge_index"])
    w = _weights(inp)

    TOTP = max(plan["TOT"], 256)
    for c, co in enumerate(plan["cores"]):
        abase = GPC * NG * c
        afp = np.zeros((NATOMP, H), np.float32)
        afp[:NATOM] = af[abase:abase + NATOM]
        co["af_pad"] = afp
        stp = np.zeros((128, TOTP // 16), np.int16)
        stp[:, :co["src_tab"].shape[1]] = co["src_tab"]
        co["src_tab_pad"] = stp

    nc = _build(plan, w)
    kernel.last_nc = nc
    in_maps = [_in_map(plan, w, c) for c in range(NCORES)]
    if int(os.environ.get("KSIM", "0")):
        res = _run_sim(nc, in_maps)
    else:
        import time
        run, unpack = _pjrt_runner(nc, in_maps)
        out = run()  # compile + first exec
        iters = int(os.environ.get("KTIME_ITERS", "5"))
        times = []
        for _ in range(iters):
            t0 = time.perf_counter()
            run()
            times.append(time.perf_counter() - t0)
        res = _SimRes()
        res.results = unpack(out)
        res.exec_time_ns = int(min(times) * 1e9) if times else None
        res.all_times_ns = [int(t * 1e9) for t in times]
    kernel.last_results = res

    # unshard
    x_full = np.zeros((E, D), np.float32)
    gfeat = np.zeros((B, D), np.float32)
    spansum = [sum(ln for (g, s, ln) in plan["spans"] if g == gg)
               for gg in range(GPC)]
    for c, co in enumerate(plan["cores"]):
        xT = res.results[c]["x_out"]
        xp = np.ascontiguousarray(xT.T)
        real = co["pad_of"] >= 0
        x_core = np.zeros((co["nb"], D), np.float32)
        x_core[co["pad_of"][real]] = xp[real]
        x_full[co["b0"]:co["b1"]] = x_core
        gfs = res.results[c]["gf_out"]          # [64, 2] span sums incl pads
        for g in range(GPC):
            cnt = co["cnt"][g]
            npadg = spansum[g] - cnt
            val = (gfs[:, g].astype(np.float64) - npadg * w["x_padval"])
            gfeat[GPC * c + g] = (val / max(cnt, 1.0)).astype(np.float32)
    return x_full, gfeat


# revision 26
# speedup vs baseline: 1.0030x; 1.0030x over previous
# Trainium2 Bass kernel for nn_BondLevel (gnn_message_passing).
#
# Sharding: data-parallel over the 16 graphs -> 2 graphs per NeuronCore,
# 8 cores, no collectives.  Per core:
#   init MLP factored per-atom:   pre0 = u[row] + v[col]; u,v = af @ iw1a/b
#   message MLP factored per-bond: pre = a[dst] + b[src],
#       a = x@W1a + emb_w[bt] + mb1  (emb_w = emb@W1c),   b = x@W1b
#   edges sorted by dst, grouped by dst-degree k, slot-major layout so the
#   segment sum is k-1 dense adds on DVE; b[src] via SWDGE dma_gather.
#   mw2 folded into uw1a on host: W_A = mw2@uw1a, bias_A = mb2@uw1a.
# Per-bond tensors live feature-major [64|65, NBP]; per-edge tensors live
# token-major (gather layout); PE transposes bridge the two.
import os
import numpy as np
import ml_dtypes

import concourse.bass as bass
import concourse.bacc as bacc
import concourse.tile as tile
import concourse.mybir as mybir
from concourse.bass_utils import run_bass_kernel_spmd

F32 = mybir.dt.float32
BF16 = mybir.dt.bfloat16
I16 = mybir.dt.int16

N, B, E, H, D, L = 30000, 16, 60000, 128, 64, 2
NCORES = 8
GPC = B // NCORES          # graphs per core
NG = N // B                # atoms per graph
NATOM = GPC * NG           # atoms per core
NATOMP = ((NATOM + 127) // 128) * 128
ZATOM = NATOM              # zero-row index in padded atom arrays


def _ceil(a, b):
    return (a + b - 1) // b


def _wrap16(idx):
    """[16, n/16] SWDGE index layout (idx t at [t%16, t//16]), replicated to
    128 partitions (8 copies, one per Q7 core)."""
    n = len(idx)
    assert n % 16 == 0
    w = np.asarray(idx, np.int16).reshape(n // 16, 16).T.copy()
    return np.ascontiguousarray(np.tile(w, (8, 1)))


_SIGMOID_MODE = bool(int(os.environ.get("KACT_SIGMOID", "0")))


def silu_np(x):
    if _SIGMOID_MODE:
        return 1.0 / (1.0 + np.exp(-x))
    return x / (1.0 + np.exp(-x))


def _plan(edge_index, bond_edge_index):
    """Common (SPMD-uniform) padded layout + per-core tables."""
    row, col = np.asarray(edge_index[0]), np.asarray(edge_index[1])
    src, dst = np.asarray(bond_edge_index[0]), np.asarray(bond_edge_index[1])
    g_bond = row // NG
    assert np.all(np.diff(g_bond) >= 0), "bonds must be sorted by graph"

    cores = []
    for c in range(NCORES):
        glo, ghi = GPC * c, GPC * (c + 1)
        b0 = int(np.searchsorted(g_bond, glo))
        b1 = int(np.searchsorted(g_bond, ghi))
        nb = b1 - b0
        esel = (dst >= b0) & (dst < b1)
        ls, ld = src[esel] - b0, dst[esel] - b0
        assert ls.min() >= 0 and ls.max() < nb, "bond edges cross graphs"
        deg = np.bincount(ld, minlength=nb)
        abase = GPC * NG * c
        cores.append(dict(b0=b0, b1=b1, nb=nb, ls=ls, ld=ld, deg=deg,
                          gb=g_bond[b0:b1] - glo,
                          r_loc=row[b0:b1] - abase, c_loc=col[b0:b1] - abase))

    kmax = max(int(co["deg"].max()) for co in cores)
    h0 = np.zeros(kmax + 1, np.int64)
    h1 = np.zeros(kmax + 1, np.int64)
    for co in cores:
        for k in range(kmax + 1):
            m = co["deg"] == k
            h0[k] = max(h0[k], int((m & (co["gb"] == 0)).sum()))
            h1[k] = max(h1[k], int((m & (co["gb"] == 1)).sum()))
    npad = np.array([_ceil(int(h0[k] + h1[k]), 128) * 128
                     for k in range(kmax + 1)])
    goff = np.concatenate([[0], np.cumsum(npad)])
    NBP = int(goff[-1])

    groups = []
    for k in range(1, kmax + 1):
        if npad[k] == 0:
            continue
        chunks = int(npad[k]) // 128
        c_sub = max(1, min(32 // k, chunks))
        while chunks % c_sub:
            c_sub -= 1
        groups.append(dict(k=k, off=int(goff[k]), chunks=chunks, c_sub=c_sub))

    for co in cores:
        nb, deg, gb = co["nb"], co["deg"], co["gb"]
        pad_of = np.full(NBP, -1, np.int64)
        pos_of = np.full(nb, -1, np.int64)
        for k in range(kmax + 1):
            for side, base in ((0, 0), (1, int(h0[k]))):
                ids = np.nonzero((deg == k) & (gb == side))[0]
                p = goff[k] + base + np.arange(len(ids))
                pad_of[p] = ids
                pos_of[ids] = p
        co["pad_of"], co["pos_of"] = pad_of, pos_of
        real = pad_of >= 0

        ridx = np.full(NBP, ZATOM, np.int64)
        cidx = np.full(NBP, ZATOM, np.int64)
        ridx[real] = co["r_loc"][pad_of[real]]
        cidx[real] = co["c_loc"][pad_of[real]]
        co["ridx_t"], co["cidx_t"] = _wrap16(ridx), _wrap16(cidx)

        dinv = np.zeros(NBP, np.float32)
        mask = np.zeros(NBP, np.float32)
        dd = deg[pad_of[real]]
        dinv[real] = 1.0 / np.maximum(dd, 1.0)
        mask[real] = (dd > 0).astype(np.float32)
        co["dinv_tok"] = np.ascontiguousarray(
            dinv.reshape(NBP // 128, 128).T).astype(np.float32)
        co["mask_row"] = np.ascontiguousarray(mask.reshape(1, NBP)).astype(ml_dtypes.bfloat16)

        order = np.argsort(co["ld"], kind="stable")
        ls_s, ld_s = co["ls"][order], co["ld"][order]
        bounds = np.searchsorted(ld_s, np.arange(nb + 1))
        tab = []
        for g in groups:
            k, off, chunks, c_sub = g["k"], g["off"], g["chunks"], g["c_sub"]
            for blk in range(chunks // c_sub):
                i0 = blk * c_sub * 128
                sub = np.full((k, c_sub * 128), NBP, np.int64)
                for t in range(c_sub * 128):
                    ob = pad_of[off + i0 + t]
                    if ob >= 0 and deg[ob] == k:
                        ss = ls_s[bounds[ob]:bounds[ob + 1]]
                        sub[:, t] = pos_of[ss]
                tab.append(sub.reshape(-1))
        tab = np.concatenate(tab) if tab else np.zeros(16, np.int64)
        co["src_tab"] = _wrap16(tab)
        co["cnt"] = np.array([(gb == 0).sum(), (gb == 1).sum()], np.float64)

    spans = []  # (graph, start, len) common across cores
    for k in range(kmax + 1):
        if npad[k] == 0:
            continue
        o = int(goff[k])
        if h0[k]:
            spans.append((0, o, int(h0[k])))
        if h1[k]:
            spans.append((1, o + int(h0[k]), int(h1[k])))
    TOT = sum(g["k"] * g["chunks"] * 128 for g in groups)
    deg0span = (int(goff[0]), int(npad[0])) if npad[0] else None
    return dict(NBP=NBP, groups=groups, spans=spans, deg0span=deg0span,
                kmax=kmax, TOT=TOT, cores=cores)


def _weights(inp):
    iw1, ib1 = np.asarray(inp["iw1"], np.float32), np.asarray(inp["ib1"], np.float32)
    iw2, ib2 = np.asarray(inp["iw2"], np.float32), np.asarray(inp["ib2"], np.float32)
    emb = np.asarray(inp["emb"], np.float32)
    mw1, mb1 = np.asarray(inp["mw1"], np.float32), np.asarray(inp["mb1"], np.float32)
    mw2, mb2 = np.asarray(inp["mw2"], np.float32), np.asarray(inp["mb2"], np.float32)
    uw1, ub1 = np.asarray(inp["uw1"], np.float32), np.asarray(inp["ub1"], np.float32)
    uw2, ub2 = np.asarray(inp["uw2"], np.float32), np.asarray(inp["ub2"], np.float32)

    w = {}
    w["rhs_uv"] = np.ascontiguousarray(
        np.concatenate([iw1[:H], iw1[H:]], axis=1))          # [128, 128]
    w["ib1_half"] = np.concatenate([ib1 / 2, ib1 / 2]).reshape(1, 2 * D)
    w["has_ib1"] = bool(np.any(ib1 != 0))
    w["lhsT_iw2"] = np.concatenate([iw2, np.zeros((1, D), np.float32)]).astype(ml_dtypes.bfloat16)
    w["ib2"] = np.ascontiguousarray(ib2.reshape(D, 1))
    te_tab = np.zeros((8, 2 * D), np.float32)
    rhs_ab, lhsT_WA, lhsT_uw1b, lhsT_uw2, lhsT_I = [], [], [], [], []
    for l in range(L):
        W1a, W1b, W1c = mw1[l][:D], mw1[l][D:2 * D], mw1[l][2 * D:]
        te_tab[:5, l * D:(l + 1) * D] = emb[l] @ W1c
        ra = np.zeros((D + 1, 2 * D), np.float32)
        ra[:D, :D], ra[D, :D] = W1a, mb1[l]
        ra[:D, D:] = W1b
        rhs_ab.append(ra)
        WA = mw2[l] @ uw1[l][:D]
        bA = (mb2[l] @ uw1[l][:D]).reshape(1, D)
        lhsT_WA.append(np.concatenate([WA, bA]))
        lhsT_uw1b.append(np.concatenate([uw1[l][D:], np.zeros((1, D), np.float32)]))
        lhsT_uw2.append(np.concatenate([uw2[l], np.zeros((1, D), np.float32)]))
        lhsT_I.append(np.concatenate([np.eye(D, dtype=np.float32),
                                      np.zeros((1, D), np.float32)]))
    w["te_tab"] = te_tab.astype(ml_dtypes.bfloat16)
    w["rhs_ab"] = np.ascontiguousarray(np.stack(rhs_ab))
    w["lhsT_WA"] = np.ascontiguousarray(np.stack(lhsT_WA)).astype(ml_dtypes.bfloat16)
    w["lhsT_uw1b"] = np.ascontiguousarray(np.stack(lhsT_uw1b))
    w["lhsT_uw2"] = np.ascontiguousarray(np.stack(lhsT_uw2)).astype(ml_dtypes.bfloat16)
    w["lhsT_I"] = np.ascontiguousarray(np.stack(lhsT_I))
    w["ub1"] = np.ascontiguousarray(ub1.reshape(L, D, 1))
    w["ub2"] = np.ascontiguousarray(ub2.reshape(L, D, 1))
    w["ident"] = np.eye(128, dtype=np.float32)

    # value of padded x columns per layer end (depends only on biases)
    xp = silu_np(np.zeros(D, np.float32) + ib1) @ iw2 + ib2
    for l in range(L):
        hp = silu_np(xp @ uw1[l][D:] + ub1[l])
        xp = xp + hp @ uw2[l] + ub2[l]
    w["x_padval"] = xp.astype(np.float32).astype(np.float64)
    return w


def _build(plan, w):
    NBP = plan["NBP"]
    CB = NBP // 128
    TOT = plan["TOT"]
    TOTP = max(TOT, 256)
    sweeps = [(s, min(512, NBP - s)) for s in range(0, NBP, 512)]
    nc = bacc.Bacc("TRN2", target_bir_lowering=False, debug=False,
                   num_devices=NCORES)

    def din(name, shape, dt):
        return nc.dram_tensor(name, list(shape), dt, kind="ExternalInput")

    i_af = din("af", (NATOMP, H), F32)
    i_ridx = din("ridx", (128, NBP // 16), I16)
    i_cidx = din("cidx", (128, NBP // 16), I16)
    i_stab = din("stab", (128, TOTP // 16), I16)
    i_dinv = din("dinv", (128, CB), F32)
    i_mask = din("maskrow", (1, NBP), BF16)
    i_rhs_uv = din("rhs_uv", (H, 2 * D), F32)
    i_lhsT_iw2 = din("lhsT_iw2", (D + 1, D), BF16)
    i_ib2 = din("ib2", (D, 1), F32)
    i_te_tab = din("te_tab", (8, 2 * D), BF16)
    i_rhs_ab = din("rhs_ab", (L, D + 1, 2 * D), F32)
    i_lhsT_WA = din("lhsT_WA", (L, D + 1, D), BF16)
    i_lhsT_uw1b = din("lhsT_uw1b", (L, D + 1, D), F32)
    i_lhsT_uw2 = din("lhsT_uw2", (L, D + 1, D), BF16)
    i_lhsT_I = din("lhsT_I", (L, D + 1, D), F32)
    i_ub1 = din("ub1", (L, D, 1), F32)
    i_ub2 = din("ub2", (L, D, 1), F32)
    i_ident = din("ident", (128, 128), F32)
    if w["has_ib1"]:
        i_ib1h = din("ib1_half", (1, 2 * D), F32)

    o_x = nc.dram_tensor("x_out", [D, NBP], F32, kind="ExternalOutput")
    o_gf = nc.dram_tensor("gf_out", [D, GPC], F32, kind="ExternalOutput")

    SILU = (mybir.ActivationFunctionType.Sigmoid if _SIGMOID_MODE
            else mybir.ActivationFunctionType.Silu)
    IDENT = mybir.ActivationFunctionType.Identity
    SQRT = mybir.ActivationFunctionType.Sqrt
    ALU = mybir.AluOpType
    AX = mybir.AxisListType
    BLK = 8

    with tile.TileContext(nc) as tc:
        with (
            tc.tile_pool(name="persist", bufs=1) as pp,
            tc.tile_pool(name="weights", bufs=1) as wp,
            tc.tile_pool(name="psA", bufs=2, space="PSUM") as psA,
            tc.tile_pool(name="psT", bufs=2, space="PSUM") as psT,
            tc.tile_pool(name="psU", bufs=3, space="PSUM") as psU,
            tc.tile_pool(name="dram", bufs=1, space="DRAM") as dp,
        ):
            x_T = pp.tile([D + 1, NBP], F32, tag="x_T")
            s_T = pp.tile([D + 1, NBP], BF16, tag="s_T")
            a_tok = pp.tile([128, CB, D], BF16, tag="a_tok")
            te_sel = pp.tile([128, CB, 2 * D], BF16, tag="te_sel")
            stab = pp.tile([128, TOTP // 16], I16, tag="stab")
            ridx = pp.tile([128, NBP // 16], I16, tag="ridx")
            cidx = pp.tile([128, NBP // 16], I16, tag="cidx")
            te_idx = pp.tile([128, NBP // 16], I16, tag="te_idx")
            dinv = pp.tile([128, CB], F32, tag="dinv")
            ident = pp.tile([128, 128], F32, tag="ident")
            btf = pp.tile([128, CB], F32, tag="btf")
            bti = pp.tile([128, CB], I16, tag="bti")
            sdot = pp.tile([128, CB], F32, tag="sdot")
            ni2 = pp.tile([128, CB], F32, tag="ni2")
            nj2 = pp.tile([128, CB], F32, tag="nj2")
            partials = pp.tile([D, 64], F32, tag="partials")

            rhs_uv = wp.tile([H, 2 * D], F32, tag="w0")
            lhsT_iw2 = wp.tile([D + 1, D], BF16, tag="w1")
            ib2 = wp.tile([D, 1], F32, tag="w2")
            rhs_ab = [wp.tile([D + 1, 2 * D], F32, tag=f"wab{l}", name=f"rhs_ab{l}") for l in range(L)]
            lhsT_WA = [wp.tile([D + 1, D], BF16, tag=f"wWA{l}", name=f"lhsT_WA{l}") for l in range(L)]
            lhsT_uw1b = [wp.tile([D + 1, D], F32, tag=f"wu1{l}", name=f"lhsT_uw1b{l}") for l in range(L)]
            lhsT_uw2 = [wp.tile([D + 1, D], BF16, tag=f"wu2{l}", name=f"lhsT_uw2{l}") for l in range(L)]
            lhsT_I = [wp.tile([D + 1, D], F32, tag=f"wI{l}", name=f"lhsT_I{l}") for l in range(L)]
            ub1 = [wp.tile([D, 1], F32, tag=f"b1{l}", name=f"ub1_{l}") for l in range(L)]
            ub2 = [wp.tile([D, 1], F32, tag=f"b2{l}", name=f"ub2_{l}") for l in range(L)]

            for t, srcap in ((rhs_uv, i_rhs_uv), (lhsT_iw2, i_lhsT_iw2),
                             (ib2, i_ib2), (ident, i_ident), (stab, i_stab),
                             (ridx, i_ridx), (cidx, i_cidx), (dinv, i_dinv)):
                nc.sync.dma_start(t[:], srcap[:])
            for l in range(L):
                for t, srcap in ((rhs_ab[l], i_rhs_ab), (lhsT_WA[l], i_lhsT_WA),
                                 (lhsT_uw1b[l], i_lhsT_uw1b),
                                 (lhsT_uw2[l], i_lhsT_uw2), (lhsT_I[l], i_lhsT_I),
                                 (ub1[l], i_ub1), (ub2[l], i_ub2)):
                    nc.sync.dma_start(t[:], srcap[l])
            nc.sync.dma_start(s_T[D:D + 1, :], i_mask[:])
            nc.vector.memset(x_T[D:D + 1, :], 1.0)

            ubuf = dp.tile([NATOMP, D], F32, tag="ubuf")
            vbuf = dp.tile([NATOMP, D], F32, tag="vbuf")
            bbuf = dp.tile([NBP + 16, D], F32, tag="bbuf")
            te_hbm = dp.tile([8, 2 * D], BF16, tag="tehbm")
            nc.sync.dma_start(te_hbm[:], i_te_tab[:])

            # ---- prologue --------------------------------------------------
            ACH = NATOMP // 128
            if w["has_ib1"]:
                ib1h = wp.tile([1, 2 * D], F32, tag="ib1h")
                nc.sync.dma_start(ib1h[:], i_ib1h[:])
                ones_row = wp.tile([1, 128], F32, tag="ones_row")
                nc.vector.memset(ones_row[:], 1.0)
            # af -> af_T (PE transpose, 128-chunks), then u/v and write out
            p1 = tc.alloc_tile_pool(name="prolog", bufs=1)
            p2 = tc.alloc_tile_pool(name="prolog2", bufs=2)
            af_T = p1.tile([H, NATOMP], F32, tag="af_T")
            for b0 in range(0, ACH, BLK):
                nch = min(BLK, ACH - b0)
                afc = p2.tile([128, BLK, H], F32, tag="afc")
                nc.sync.dma_start(
                    afc[:, 0:nch, :],
                    i_af.ap().rearrange("(c p) f -> p c f", p=128)[:, b0:b0 + nch, :])
                for t0 in range(0, nch, 4):
                    nb4 = min(4, nch - t0)
                    pt = psT.tile([128, 512], F32, tag="ptb")
                    for i in range(nb4):
                        nc.tensor.transpose(pt[:, i * 128:(i + 1) * 128],
                                            afc[:, t0 + i, :], ident[:])
                    nc.scalar.activation(
                        af_T[:, (b0 + t0) * 128:(b0 + t0 + nb4) * 128],
                        pt[:, 0:nb4 * 128], IDENT)
                uvs = p2.tile([128, BLK, 2 * D], F32, tag="uvs")
                for t in range(nch):
                    cix = b0 + t
                    pab = psA.tile([128, 2 * D], F32, tag="pab")
                    nc.tensor.matmul(pab[:], af_T[:, cix * 128:(cix + 1) * 128],
                                     rhs_uv[:], start=True, stop=not w["has_ib1"])
                    if w["has_ib1"]:
                        nc.tensor.matmul(pab[:], ones_row[:], ib1h[:],
                                         start=False, stop=True)
                    nc.scalar.activation(uvs[:, t, :], pab[:], IDENT)
                nc.sync.dma_start(
                    ubuf[:, :].rearrange("(c p) f -> p c f", p=128)[:, b0:b0 + nch, :],
                    uvs[:, 0:nch, 0:D])
                nc.sync.dma_start(
                    vbuf[:, :].rearrange("(c p) f -> p c f", p=128)[:, b0:b0 + nch, :],
                    uvs[:, 0:nch, D:2 * D])

            p2.release()
            p1.release()
            gp = tc.alloc_tile_pool(name="gath", bufs=3)
            gpp = tc.alloc_tile_pool(name="gathp", bufs=2)
            sp = tc.alloc_tile_pool(name="stage", bufs=3)
            zrow = sp.tile([16, D], F32, tag="zrow")
            nc.vector.memset(zrow[:], 0.0)
            nc.sync.dma_start(bbuf[NBP:NBP + 16, :], zrow[:])

            # sim dot products + init-MLP gathers per 8-chunk block
            for blk in range(_ceil(CB, BLK)):
                c0 = blk * BLK
                nch = min(BLK, CB - c0)
                nidx = nch * 128
                hi = gpp.tile([128, BLK, H], F32, tag="gh0")
                hj = gpp.tile([128, BLK, H], F32, tag="gh1")
                nc.gpsimd.dma_gather(hi[:, 0:nch, :], i_af[:],
                                     ridx[:, c0 * 8:(c0 + nch) * 8],
                                     nidx, nidx, H, elem_step=H)
                nc.gpsimd.dma_gather(hj[:, 0:nch, :], i_af[:],
                                     cidx[:, c0 * 8:(c0 + nch) * 8],
                                     nidx, nidx, H, elem_step=H)
                scr = sp.tile([128, H], F32, tag="scr")
                for t in range(nch):
                    cc = c0 + t
                    nc.vector.tensor_tensor_reduce(
                        scr[:], hi[:, t, :], hj[:, t, :], 1.0, 0.0,
                        ALU.mult, ALU.add, sdot[:, cc:cc + 1])
                    nc.vector.tensor_tensor_reduce(
                        scr[:], hi[:, t, :], hi[:, t, :], 1.0, 0.0,
                        ALU.mult, ALU.add, ni2[:, cc:cc + 1])
                    nc.vector.tensor_tensor_reduce(
                        scr[:], hj[:, t, :], hj[:, t, :], 1.0, 0.0,
                        ALU.mult, ALU.add, nj2[:, cc:cc + 1])
                gu = gpp.tile([128, BLK, D], F32, tag="gg0")
                gv = gpp.tile([128, BLK, D], F32, tag="gg1")
                nc.gpsimd.dma_gather(gu[:, 0:nch, :], ubuf[:],
                                     ridx[:, c0 * 8:(c0 + nch) * 8],
                                     nidx, nidx, D, elem_step=D)
                nc.gpsimd.dma_gather(gv[:, 0:nch, :], vbuf[:],
                                     cidx[:, c0 * 8:(c0 + nch) * 8],
                                     nidx, nidx, D, elem_step=D)
                nc.vector.tensor_add(gu[:, 0:nch, :], gu[:, 0:nch, :],
                                     gv[:, 0:nch, :])
                nc.scalar.activation(gu[:, 0:nch, :], gu[:, 0:nch, :], SILU)
                for t in range(nch):
                    cc = c0 + t
                    pt2 = psT.tile([128, 128], F32, tag="pt")
                    nc.tensor.transpose(pt2[0:D, :], gu[:, t, 0:D], ident[:])
                    nc.scalar.activation(
                        s_T[0:D, cc * 128:(cc + 1) * 128], pt2[0:D, :], IDENT)

            # sim -> bt -> te_idx -> te gather
            nc.scalar.activation(ni2[:], ni2[:], SQRT)
            nc.scalar.activation(nj2[:], nj2[:], SQRT)
            nc.vector.tensor_scalar_max(ni2[:], ni2[:], 1e-8)
            nc.vector.tensor_scalar_max(nj2[:], nj2[:], 1e-8)
            nc.vector.tensor_mul(ni2[:], ni2[:], nj2[:])
            nc.vector.reciprocal(ni2[:], ni2[:])
            nc.vector.tensor_mul(sdot[:], sdot[:], ni2[:])   # sim
            is1, is2 = ni2, nj2
            nc.vector.tensor_scalar(is1[:], sdot[:], 0.8, None, ALU.is_gt)
            nc.vector.tensor_scalar(is2[:], sdot[:], 0.9, None, ALU.is_gt)
            nc.vector.tensor_scalar(btf[:], sdot[:], 0.3, None, ALU.is_lt)
            nc.vector.tensor_add(is1[:], is1[:], is2[:])
            nc.vector.tensor_scalar(is2[:], btf[:], -1.0, 1.0, ALU.mult, ALU.add)
            nc.vector.tensor_mul(is1[:], is1[:], is2[:])
            nc.vector.tensor_scalar(btf[:], btf[:], 3.0, None, ALU.mult)
            nc.vector.tensor_add(btf[:], btf[:], is1[:])
            nc.vector.tensor_copy(bti[:], btf[:])
            for r in range(8):
                nc.sync.dma_start(te_idx[0:16, r::8], bti[r * 16:(r + 1) * 16, :])
            for m in range(1, 8):
                nc.sync.dma_start(te_idx[m * 16:(m + 1) * 16, :], te_idx[0:16, :])
            half_c = CB // 2
            nc.gpsimd.dma_gather(te_sel[:, 0:half_c, :], te_hbm[:],
                                 te_idx[:, 0:half_c * 8], half_c * 128,
                                 half_c * 128, 2 * D, elem_step=2 * D)
            nc.gpsimd.dma_gather(te_sel[:, half_c:CB, :], te_hbm[:],
                                 te_idx[:, half_c * 8:], NBP - half_c * 128,
                                 NBP - half_c * 128, 2 * D, elem_step=2 * D)

            # x0 = silu(pre0) @ iw2 + ib2
            for s0, ln in sweeps:
                sl = slice(s0, s0 + ln)
                pu = psU.tile([D, 512], F32, tag="pu")
                nc.tensor.matmul(pu[0:D, 0:ln], lhsT_iw2[:], s_T[:, sl],
                                 start=True, stop=True)
                nc.scalar.activation(x_T[0:D, sl], pu[0:D, 0:ln], IDENT,
                                     bias=ib2[:])

            # ---- layers ----------------------------------------------------
            for l in range(L):
                for b0 in range(0, CB, BLK):
                    nch = min(BLK, CB - b0)
                    bst = sp.tile([128, BLK, D], F32, tag="bst")
                    for t in range(nch):
                        cix = b0 + t
                        pab = psA.tile([128, 2 * D], F32, tag="pab")
                        nc.tensor.matmul(pab[:],
                                         x_T[:, cix * 128:(cix + 1) * 128],
                                         rhs_ab[l][:], start=True, stop=True)
                        nc.vector.tensor_add(a_tok[:, cix, :], pab[:, 0:D],
                                             te_sel[:, cix, l * D:(l + 1) * D])
                        nc.scalar.activation(bst[:, t, :], pab[:, D:2 * D], IDENT)
                    nc.sync.dma_start(
                        bbuf[0:NBP, :].rearrange("(c p) f -> p c f",
                                                 p=128)[:, b0:b0 + nch, :],
                        bst[:, 0:nch, :])

                if plan["deg0span"] is not None:
                    o0, n0 = plan["deg0span"]
                    nc.vector.memset(s_T[0:D, o0:o0 + n0], 0.0)

                toff = 0
                for g in plan["groups"]:
                    k, off, chunks, c_sub = (g["k"], g["off"], g["chunks"],
                                             g["c_sub"])
                    for blk in range(chunks // c_sub):
                        nidx = k * c_sub * 128
                        gch = off // 128 + blk * c_sub
                        tg = gp.tile([128, 32, D], F32, tag="gath")
                        tgv = tg[:, 0:k * c_sub, :]
                        nc.gpsimd.dma_gather(
                            tgv, bbuf[:],
                            stab[:, toff // 16:(toff + nidx) // 16],
                            nidx, nidx, D, elem_step=D)
                        toff += nidx
                        for j in range(k):
                            nc.vector.tensor_add(
                                tg[:, j * c_sub:(j + 1) * c_sub, :],
                                tg[:, j * c_sub:(j + 1) * c_sub, :],
                                a_tok[:, gch:gch + c_sub, :])
                        nc.scalar.activation(tgv, tgv, SILU)
                        for j in range(1, k):
                            nc.vector.tensor_add(
                                tg[:, 0:c_sub, :], tg[:, 0:c_sub, :],
                                tg[:, j * c_sub:(j + 1) * c_sub, :])
                        for cc in range(c_sub):
                            nc.vector.tensor_scalar_mul(
                                tg[:, cc, :], tg[:, cc, :],
                                dinv[:, gch + cc:gch + cc + 1])
                            pt3 = psT.tile([128, 128], F32, tag="pt")
                            nc.tensor.transpose(pt3[0:D, :], tg[:, cc, 0:D],
                                                ident[:])
                            nc.scalar.activation(
                                s_T[0:D, (gch + cc) * 128:(gch + cc + 1) * 128],
                                pt3[0:D, :], IDENT)

                for s0, ln in sweeps:
                    sl = slice(s0, s0 + ln)
                    ph = psU.tile([D, 512], F32, tag="pu")
                    nc.tensor.matmul(ph[0:D, 0:ln], lhsT_WA[l][:], s_T[:, sl],
                                     start=True, stop=False)
                    nc.tensor.matmul(ph[0:D, 0:ln], lhsT_uw1b[l][:], x_T[:, sl],
                                     start=False, stop=True)
                    nc.scalar.activation(s_T[0:D, sl], ph[0:D, 0:ln], SILU,
                                         bias=ub1[l][:])
                    px = psU.tile([D, 512], F32, tag="pu")
                    nc.tensor.matmul(px[0:D, 0:ln], lhsT_uw2[l][:], s_T[:, sl],
                                     start=True, stop=False)
                    nc.tensor.matmul(px[0:D, 0:ln], lhsT_I[l][:], x_T[:, sl],
                                     start=False, stop=True)
                    nc.scalar.activation(x_T[0:D, sl], px[0:D, 0:ln], IDENT,
                                         bias=ub2[l][:])

            # ---- epilogue: pooling span sums + outputs --------------------
            nc.sync.dma_start(o_x.ap(), x_T[0:D, :])
            g0 = [(s, ln) for (g, s, ln) in plan["spans"] if g == 0]
            g1 = [(s, ln) for (g, s, ln) in plan["spans"] if g == 1]
            assert len(g0) <= 32 and len(g1) <= 32
            for i, (s, ln) in enumerate(g0):
                nc.vector.reduce_sum(partials[:, i:i + 1], x_T[0:D, s:s + ln],
                                     axis=AX.X)
            for i, (s, ln) in enumerate(g1):
                nc.vector.reduce_sum(partials[:, 32 + i:33 + i],
                                     x_T[0:D, s:s + ln], axis=AX.X)
            gf = sp.tile([D, GPC], F32, tag="gf")
            nc.vector.reduce_sum(gf[:, 0:1], partials[:, 0:len(g0)], axis=AX.X)
            nc.vector.reduce_sum(gf[:, 1:2], partials[:, 32:32 + len(g1)],
                                 axis=AX.X)
            nc.sync.dma_start(o_gf.ap(), gf[:])
            sp.release()
            gpp.release()
            gp.release()

    nc.compile()
    return nc


def _in_map(plan, w, core):
    co = plan["cores"][core]
    m = {
        "af": co["af_pad"], "ridx": co["ridx_t"], "cidx": co["cidx_t"],
        "stab": co["src_tab_pad"], "dinv": co["dinv_tok"],
        "maskrow": co["mask_row"],
        "rhs_uv": w["rhs_uv"], "lhsT_iw2": w["lhsT_iw2"], "ib2": w["ib2"],
        "te_tab": w["te_tab"], "rhs_ab": w["rhs_ab"],
        "lhsT_WA": w["lhsT_WA"], "lhsT_uw1b": w["lhsT_uw1b"],
        "lhsT_uw2": w["lhsT_uw2"], "lhsT_I": w["lhsT_I"],
        "ub1": w["ub1"], "ub2": w["ub2"], "ident": w["ident"],
    }
    if w["has_ib1"]:
        m["ib1_half"] = w["ib1_half"]
    return {k: np.ascontiguousarray(v) for k, v in m.items()}


class _SimRes:
    pass


def _pjrt_runner(nc, in_maps):
    """Build a reusable sharded PJRT callable for timing loops (the
    axon NTFF hook is unavailable in this container, so exec time is
    measured as steady-state wall time of the compiled executable)."""
    import jax
    import jax.numpy as jnp
    from jax.sharding import Mesh, PartitionSpec
    from jax.experimental.shard_map import shard_map
    from concourse import bass2jax
    import concourse.mybir as mybir
    bass2jax.install_neuronx_cc_hook()
    n_cores = len(in_maps)
    partition_name = (nc.partition_id_tensor.name
                      if nc.partition_id_tensor else None)
    in_names, out_names, out_avals, zero_outs = [], [], [], []
    for alloc in nc.m.functions[0].allocations:
        if not isinstance(alloc, mybir.MemoryLocationSet):
            continue
        name = alloc.memorylocations[0].name
        if alloc.kind == "ExternalInput":
            if name != partition_name:
                in_names.append(name)
        elif alloc.kind == "ExternalOutput":
            shape = tuple(alloc.tensor_shape)
            dtype = mybir.dt.np(alloc.dtype)
            out_names.append(name)
            out_avals.append(jax.core.ShapedArray(shape, dtype))
            zero_outs.append(np.zeros(shape, dtype))
    n_params = len(in_names)
    n_outs = len(out_avals)
    all_names = list(in_names) + list(out_names)
    if partition_name is not None:
        all_names.append(partition_name)

    def _body(*args):
        operands = list(args)
        if partition_name is not None:
            operands.append(bass2jax.partition_id_tensor())
        outs = bass2jax._bass_exec_p.bind(
            *operands, out_avals=tuple(out_avals), in_names=tuple(all_names),
            out_names=tuple(out_names), lowering_input_output_aliases=(),
            sim_require_finite=True, sim_require_nnan=True, nc=nc)
        return tuple(outs)

    devices = jax.devices()[:n_cores]
    mesh = Mesh(np.asarray(devices), ("core",))
    sharded = jax.jit(
        shard_map(_body, mesh=mesh,
                  in_specs=(PartitionSpec("core"),) * (n_params + n_outs),
                  out_specs=(PartitionSpec("core"),) * n_outs,
                  check_rep=False),
        keep_unused=True)
    from jax.sharding import NamedSharding
    shard = NamedSharding(mesh, PartitionSpec("core"))
    concat_in = [jax.device_put(
                     np.concatenate([np.asarray(in_maps[c][in_names[i]])
                                     for c in range(n_cores)], axis=0), shard)
                 for i in range(n_params)]
    concat_zeros = [jax.device_put(
                        np.zeros((n_cores * z.shape[0], *z.shape[1:]), z.dtype),
                        shard)
                    for z in zero_outs]
    jax.block_until_ready(concat_in)
    jax.block_until_ready(concat_zeros)

    nrep = int(os.environ.get("KNREP", "1"))
    if nrep > 1:
        # chain a scalar data-dependency through reps so XLA can't CSE or
        # parallelize the repeated NEFF executions
        def _body_n(*args):
            ins = list(args[:n_params])
            zouts = list(args[n_params:])
            outs = None
            for _ in range(nrep):
                operands = list(ins) + list(zouts)
                if partition_name is not None:
                    operands.append(bass2jax.partition_id_tensor())
                outs = bass2jax._bass_exec_p.bind(
                    *operands, out_avals=tuple(out_avals),
                    in_names=tuple(all_names), out_names=tuple(out_names),
                    lowering_input_output_aliases=(),
                    sim_require_finite=True, sim_require_nnan=True, nc=nc)
                ins[0] = ins[0] + (0.0 * outs[0].reshape(-1)[0]).astype(
                    ins[0].dtype)
            return tuple(outs)

        sharded_n = jax.jit(
            shard_map(_body_n, mesh=mesh,
                      in_specs=(PartitionSpec("core"),) * (n_params + n_outs),
                      out_specs=(PartitionSpec("core"),) * n_outs,
                      check_rep=False),
            keep_unused=True)

        def run():
            out = sharded_n(*concat_in, *concat_zeros)
            jax.block_until_ready(out)
            return out
    else:
        def run():
            out = sharded(*concat_in, *concat_zeros)
            jax.block_until_ready(out)
            return out

    def unpack(out_arrs):
        return [{name: np.asarray(out_arrs[i]).reshape(
                    n_cores, *out_avals[i].shape)[c]
                 for i, name in enumerate(out_names)}
                for c in range(n_cores)]

    return run, unpack


def _run_sim(nc, in_maps):
    from concourse.bass_interp import CoreSim
    results = []
    ncse = int(os.environ.get("KSIM_CORES", str(NCORES)))
    for m in in_maps[:ncse]:
        sim = CoreSim(nc, trace=False)
        for k, v in m.items():
            sim.tensor(k)[:] = v
        sim.simulate()
        results.append({o: np.array(sim.tensor(o))
                        for o in ("x_out", "gf_out")})
    while len(results) < NCORES:
        results.append(results[-1])
    r = _SimRes()
    r.results = results
    r.exec_time_ns = None
    return r


def kernel(**inputs):
    inp = {k: np.asarray(v) for k, v in inputs.items()}
    af = np.asarray(inp["atom_features"], np.float32)

    plan = _plan(inp["edge_index"], inp["bond_edge_index"])
    w = _weights(inp)

    TOTP = max(plan["TOT"], 256)
    for c, co in enumerate(plan["cores"]):
        abase = GPC * NG * c
        afp = np.zeros((NATOMP, H), np.float32)
        afp[:NATOM] = af[abase:abase + NATOM]
        co["af_pad"] = afp
        stp = np.zeros((128, TOTP // 16), np.int16)
        stp[:, :co["src_tab"].shape[1]] = co["src_tab"]
        co["src_tab_pad"] = stp

    nc = _build(plan, w)
    kernel.last_nc = nc
    in_maps = [_in_map(plan, w, c) for c in range(NCORES)]
    if int(os.environ.get("KSIM", "0")):
        res = _run_sim(nc, in_maps)
    else:
        import time
        run, unpack = _pjrt_runner(nc, in_maps)
        out = run()  # compile + first exec
        iters = int(os.environ.get("KTIME_ITERS", "5"))
        times = []
        for _ in range(iters):
            t0 = time.perf_counter()
            run()
            times.append(time.perf_counter() - t0)
        res = _SimRes()
        res.results = unpack(out)
        res.exec_time_ns = int(min(times) * 1e9) if times else None
        res.all_times_ns = [int(t * 1e9) for t in times]
    kernel.last_results = res

    # unshard
    x_full = np.zeros((E, D), np.float32)
    gfeat = np.zeros((B, D), np.float32)
    spansum = [sum(ln for (g, s, ln) in plan["spans"] if g == gg)
               for gg in range(GPC)]
    for c, co in enumerate(plan["cores"]):
        xT = res.results[c]["x_out"]
        xp = np.ascontiguousarray(xT.T)
        real = co["pad_of"] >= 0
        x_core = np.zeros((co["nb"], D), np.float32)
        x_core[co["pad_of"][real]] = xp[real]
        x_full[co["b0"]:co["b1"]] = x_core
        gfs = res.results[c]["gf_out"]          # [64, 2] span sums incl pads
        for g in range(GPC):
            cnt = co["cnt"][g]
            npadg = spansum[g] - cnt
            val = (gfs[:, g].astype(np.float64) - npadg * w["x_padval"])
            gfeat[GPC * c + g] = (val / max(cnt, 1.0)).astype(np.float32)
    return x_full, gfeat
